# revision 23
# baseline (speedup 1.0000x reference)
"""CapsuleLayer (dynamic routing) Trainium2 Bass kernel.

Math (per example b):
  u_hat[b,i,o,n] = sum_v x[b,i,v] * W[i,o,v,n]        I=1152, O=10, V=8, N=16
  b_logits = 0; repeat n_routing times:
    c = softmax_o(b_logits); s = sum_i c*u_hat; out = squash(s)
    if not last: b_logits += sum_n u_hat*out

Distribution: batch B=256 sharded over 8 cores (32 each). W replicated.

Per-core layout (chunk = 8 examples, 4 chunks), i = ib*16 + il:
  K partitions k = il*8+v   (contraction rows of the u_hat matmul)
  M partitions p = b*16+il  (rows of u_hat / routing state)
  U[c] [128, 72, 160] bf16  u_hat,  U[(b,il), ib, (o,n)]
  xbd  [128, 18, 128] bf16  block-diag x stationary quarters (2 rotating bufs)
  cbd[c] [128, 80, 72] bf16 block-diag c stationary: CBD[(b,il), (o,b'), ib]
  w2   [128, 72, 160] bf16  W2[(il,v), ib, (o,n)] = W[ib*16+il, o, v, n]
  u_hat matmul (per ib): psum[(b,il'),(o,n)] = XBD[:,ib,:].T @ w2[:,ib,:]
  s matmul (per iter): psum[(o,b'),(o',n)] += CBD[:,:,ib].T @ U[:,ib,:]
    -> diagonal o==o' holds s[b', o, n]  (extracted via small DMAs)

Schedule: phase 1 computes u_hat for ALL 4 chunks up front; routing runs as
a SOFTWARE PIPELINE over (iteration, chunk) tasks with three stages
  S(k): s-matmul (PE), PSUM evacuation (ACT), diag extraction (DMA),
        squash front  s^2 + n-tree + (1+nsq)^2*(nsq+eps)  (GPSIMD)
  A(k): rsqrt chain + squash scale + v (DVE, STT-fused Newton), v->bf16,
        vrep replication via a tiny ones-block-diag matmul (PE) + PSUM
        copy (ACT), agreement product + t8/t4 tree (DVE), t2t/final/bsum
        levels (GPSIMD), per-half exp (ACT)
  B(k): softmax o-sum tree + reciprocal + normalize (DVE), block-diag
        scatter to cbd (DMA)
emitted as S(k), A(k-1), B(k-2) so every engine's FIFO only sees work whose
inputs were produced >= 1 task-period earlier - no cross-engine stalls.

SBUF is within ~1KB of full, so scratch is carved aggressively: all squash
scalar temps live in slices of one [8, 248] tile, the softmax o-sum tree is
carved into the c2n/rs tiles via bitcast views, and the agreement t4/t2t
levels are carved back into ph's storage after it is consumed.
"""

import os
import sys

import numpy as np

_TRN_REPO = "/opt/trn_rl_repo"
if _TRN_REPO not in sys.path:
    sys.path.insert(0, _TRN_REPO)

EPS = 1e-10
B, I, V, O, N = 256, 1152, 8, 10, 16
NCORES = 8
BLOC = B // NCORES          # 32 examples per core
BC = 8                      # examples per chunk
NCHUNK = BLOC // BC         # 4
IB = I // 16                # 72 i-blocks
IBH = IB // 2               # 36 (agreement half granularity)
IBQ = IB // 4               # 18 (xbd staging granularity)
ON = O * N                  # 160
RSQRT_MAGIC = 0x5F3759DF


class _Ctx:
    pass


def _build(n_routing: int):
    import concourse.bacc as bacc
    import concourse.tile as tile
    from concourse import mybir

    nc = bacc.Bacc("TRN2", target_bir_lowering=False, debug=False)
    f32 = mybir.dt.float32
    bf16 = mybir.dt.bfloat16

    xbdh = nc.dram_tensor(
        "xbdh", [NCHUNK, 128, IB, 128], bf16, kind="ExternalInput"
    )
    w2 = nc.dram_tensor("w2", [128, IB, ON], bf16, kind="ExternalInput")
    e2 = nc.dram_tensor("e2", [128, 80], bf16, kind="ExternalInput")
    ed = nc.dram_tensor("ed", [8, 128], bf16, kind="ExternalInput")
    out_d = nc.dram_tensor("out", [BLOC, O, N], f32, kind="ExternalOutput")

    X = _Ctx()
    X.nc, X.mybir = nc, mybir
    X.f32, X.bf16 = f32, bf16
    X.i32 = mybir.dt.int32
    X.out_d = out_d
    X.n_routing = n_routing

    with tile.TileContext(nc) as tc:
        with (
            tc.tile_pool(name="state", bufs=1) as state,
            tc.tile_pool(name="small", bufs=1) as small,
            tc.tile_pool(name="tree", bufs=1) as tree,
            tc.tile_pool(name="psA", bufs=3, space="PSUM") as psA,
            tc.tile_pool(name="psS", bufs=4, space="PSUM") as psS,
        ):
            X.small, X.tree, X.psS, X.psA = small, tree, psS, psA
            X.Us = [
                state.tile([128, IB, ON], bf16, tag=f"U{j}", name=f"U{j}")
                for j in range(NCHUNK)
            ]
            X.cbds = [
                state.tile([128, 80, IB], bf16, tag=f"cbd{j}", name=f"cbd{j}")
                for j in range(NCHUNK)
            ] if n_routing > 1 else []
            for j, cb in enumerate(X.cbds):
                if j % 2 == 0:
                    nc.scalar.memzero(cb[:])
                else:
                    nc.gpsimd.memset(cb[:], 0.0)
            X.bbs = [
                state.tile([128, IB, O], bf16, tag=f"bb{j}", name=f"bb{j}")
                for j in range(NCHUNK)
            ] if n_routing > 1 else []
            X.e2s = state.tile([128, 80], bf16, name="e2s")
            nc.sync.dma_start(out=X.e2s[:], in_=e2[:])
            X.eds = state.tile([8, 128], bf16, name="eds")
            nc.sync.dma_start(out=X.eds[:], in_=ed[:])
            w2s = state.tile([128, IB, ON], bf16, name="w2s")
            for q in range(4):
                nc.scalar.dma_start(
                    out=w2s[:, q * IBQ:(q + 1) * IBQ, :],
                    in_=w2[:, q * IBQ:(q + 1) * IBQ, :],
                )
            X.v3b = state.tile([32, ON], bf16, name="v3b")
            if n_routing > 1:
                nc.vector.memset(X.v3b[:], 0.0)
            X.vrep = state.tile([128, ON], bf16, name="vrep")
            # squash-front constants (GPSIMD runs only tensor_tensor ops)
            X.csq = state.tile([BC, 2 * O], f32, name="csq")
            nc.gpsimd.memset(X.csq[:, 0 * O:1 * O], 1.0)
            nc.gpsimd.memset(X.csq[:, 1 * O:2 * O], EPS)

            # ---------------- phase 1: u_hat for all chunks ----------------
            for c in range(NCHUNK):
                for h in range(4):
                    xbd = small.tile(
                        [128, IBQ, 128], bf16, tag="xbd", bufs=2, name="xbd"
                    )
                    nc.sync.dma_start(
                        out=xbd[:], in_=xbdh[c, :, h * IBQ:(h + 1) * IBQ, :]
                    )
                    for g in range(IBQ // 3):
                        ps = psA.tile([128, 3, ON], f32, tag="psA")
                        for j in range(3):
                            ib = h * IBQ + g * 3 + j
                            nc.tensor.matmul(
                                ps[:, j, :],
                                xbd[:, g * 3 + j, :],
                                w2s[:, ib, :],
                                start=True,
                                stop=True,
                            )
                        dst = X.Us[c][
                            :, h * IBQ + g * 3:h * IBQ + (g + 1) * 3, :
                        ]
                        if g % 2 == 0:
                            nc.vector.tensor_copy(dst, ps[:])
                        else:
                            nc.scalar.copy(dst, ps[:])

            # ------- routing: software-pipelined S/A/B over (it, c) -------
            tasks = [(it, c) for it in range(n_routing) for c in range(NCHUNK)]
            ctxs = [None] * len(tasks)
            for k in range(len(tasks) + 2):
                if k < len(tasks):
                    ctxs[k] = _stage_S(X, *tasks[k])
                if 1 <= k <= len(tasks):
                    _stage_A(X, ctxs[k - 1], *tasks[k - 1])
                if k >= 2:
                    _stage_B(X, ctxs[k - 2], *tasks[k - 2])

    nc.compile()
    return nc


def _stage_S(X, it, c):
    """s-matmul (PE) -> sY (ACT) -> diag extract (DMA) -> squash front
    (GPSIMD: s^2, n-tree, (1+nsq)^2*(nsq+eps))."""
    nc, mybir = X.nc, X.mybir
    f32, N_ = X.f32, N
    g = nc.gpsimd
    OP = mybir.AluOpType
    dmaqs = [nc.sync, nc.scalar, nc.gpsimd]
    U, cbd = X.Us[c], (X.cbds[c] if X.cbds else None)

    t = _Ctx()
    pss = X.psS.tile([80, ON], f32, tag="psS")
    for ib in range(IB):
        lhsT = X.e2s[:] if it == 0 else cbd[:, :, ib]
        nc.tensor.matmul(
            pss[:], lhsT, U[:, ib, :], start=(ib == 0), stop=(ib == IB - 1)
        )
    sY = X.small.tile([80, ON], f32, tag="sY", bufs=2)
    nc.scalar.copy(sY[:], pss[:])
    t.s3 = X.small.tile([BC, ON], f32, tag="s3", bufs=2)
    for o in range(O):
        dmaqs[o % 3].dma_start(
            out=t.s3[:, o * N_:(o + 1) * N_],
            in_=sY[o * 8:(o + 1) * 8, o * N_:(o + 1) * N_],
        )
    # squash front on GPSIMD; temps carved in one scratch tile
    sw = X.small.tile([BC, 248], f32, tag="sw", bufs=2, name="sw")
    t.sw = sw
    sq = sw[:, 0:160].rearrange("b (o n) -> b o n", n=N_)
    q8 = sw[:, 160:240].rearrange("b (o n) -> b o n", n=8)
    q4 = sw[:, 0:40].rearrange("b (o n) -> b o n", n=4)
    q2 = sw[:, 40:60].rearrange("b (o n) -> b o n", n=2)
    nsq = sw[:, 60:70]
    np1 = sw[:, 70:80]
    d1 = sw[:, 80:90]
    dd = sw[:, 90:100]
    nse = sw[:, 110:120]
    s34 = t.s3[:].rearrange("b (o n) -> b o n", n=N_)
    g.tensor_mul(sq, s34, s34)
    g.tensor_add(q8, sq[:, :, 0:8], sq[:, :, 8:16])
    g.tensor_add(q4, q8[:, :, 0:4], q8[:, :, 4:8])
    g.tensor_add(q2, q4[:, :, 0:2], q4[:, :, 2:4])
    g.tensor_add(nsq, q2[:, :, 0], q2[:, :, 1])
    g.tensor_add(np1, nsq, X.csq[:, 0:O])
    g.tensor_mul(d1, np1, np1)
    g.tensor_add(nse, nsq, X.csq[:, O:2 * O])
    g.tensor_mul(dd, d1, nse)
    return t


def _stage_A(X, t, it, c):
    """DVE rsqrt chain + v3; vrep via ones-matmul (PE) + copy (ACT);
    agreement products + t8/t4 (DVE); t2t/final/bsum (GPSIMD); exp (ACT)."""
    nc, mybir = X.nc, X.mybir
    f32, bf16, i32, N_ = X.f32, X.bf16, X.i32, N
    OP = mybir.AluOpType
    AF = mybir.ActivationFunctionType
    v = nc.vector
    g = nc.gpsimd
    U, bb = X.Us[c], (X.bbs[c] if X.bbs else None)
    last = it == X.n_routing - 1

    sw = t.sw
    nsq = sw[:, 60:70]
    dd = sw[:, 90:100]
    ya = sw[:, 100:110]
    y2 = sw[:, 110:120]     # nse slot, consumed
    w1 = sw[:, 120:130]
    yb = sw[:, 140:150]
    v.tensor_scalar(
        ya.bitcast(i32), dd.bitcast(i32), 1, None,
        op0=OP.logical_shift_right,
    )
    v.tensor_scalar(
        ya.bitcast(i32), ya.bitcast(i32), -1, RSQRT_MAGIC,
        op0=OP.mult, op1=OP.add,
    )
    yy, yn = ya, yb
    for _ in range(2):
        v.tensor_mul(y2, yy, yy)
        v.scalar_tensor_tensor(w1, y2, -0.5, dd, op0=OP.mult, op1=OP.mult)
        v.scalar_tensor_tensor(yn, w1, 1.5, yy, op0=OP.add, op1=OP.mult)
        yy, yn = yn, yy
    sc = y2
    v.tensor_mul(sc, nsq, yy)
    v3 = X.small.tile([BC, ON], f32, tag="v3", bufs=2)
    v.tensor_mul(
        v3[:].rearrange("b (o n) -> b o n", n=N_),
        t.s3[:].rearrange("b (o n) -> b o n", n=N_),
        sc.unsqueeze(2).broadcast_to([BC, O, N_]),
    )

    if last:
        nc.scalar.dma_start(
            out=X.out_d[c * BC:(c + 1) * BC, :, :],
            in_=v3[:].rearrange("b (o n) -> b o n", n=N_),
        )
        return

    # vrep[(b,il), (o,n)] = v3[b, (o,n)] via ones-block-diag matmul
    v.tensor_copy(X.v3b[0:BC, :], v3[:])
    psv = X.psA.tile([128, ON], f32, tag="psv", bufs=1, name="psv")
    nc.tensor.matmul(psv[:], X.eds[:], X.v3b[0:BC, :], start=True, stop=True)
    nc.scalar.copy(X.vrep[:], psv[:])

    # agreement product + tree; t2t/final/bsum levels go to GPSIMD
    t.bcur = bb if it == 0 else X.small.tile(
        [128, IB, O], bf16, tag="bsum", bufs=2, name="bsum"
    )
    t.c2 = X.small.tile([128, O, IB], bf16, tag="c2", bufs=2, name="c2")
    for h in range(2):
        ph = X.tree.tile([128, IBH * ON], bf16, tag="ph", name="ph")
        phv = ph[:].rearrange("p (i o n) -> p i o n", o=O, n=N_)
        t4v = ph[:, 0:IBH * O * 4].rearrange(
            "p (i o n) -> p i o n", o=O, n=4
        )
        t2v = ph[:, IBH * O * 4:IBH * O * 6].rearrange(
            "p (i o n) -> p i o n", o=O, n=2
        )
        afv = ph[:, IBH * O * 6:IBH * O * 7].rearrange(
            "p (i o) -> p i o", o=O
        )
        v.tensor_mul(
            phv,
            U[:, h * IBH:(h + 1) * IBH, :].rearrange(
                "p i (o n) -> p i o n", n=N_
            ),
            X.vrep[:]
            .rearrange("p (o n) -> p o n", n=N_)
            .unsqueeze(1)
            .broadcast_to([128, IBH, O, N_]),
        )
        t8 = X.tree.tile([128, IBH, O, 8], bf16, tag="t8", name="t8")
        v.tensor_add(t8[:], phv[:, :, :, 0:8], phv[:, :, :, 8:16])
        v.tensor_add(t4v, t8[:, :, :, 0:4], t8[:, :, :, 4:8])
        g.tensor_add(t2v, t4v[:, :, :, 0:2], t4v[:, :, :, 2:4])
        bslice = t.bcur[:, h * IBH:(h + 1) * IBH, :]
        if it == 0:
            g.tensor_add(bslice, t2v[:, :, :, 0], t2v[:, :, :, 1])
        else:
            g.tensor_add(afv, t2v[:, :, :, 0], t2v[:, :, :, 1])
            g.tensor_add(bslice, afv, bb[:, h * IBH:(h + 1) * IBH, :])
        # exp of this half on ACT as soon as its logits land
        nc.scalar.activation(
            t.c2[:, :, h * IBH:(h + 1) * IBH].transpose([0, 2, 1]),
            bslice, AF.Exp,
        )
    if it != 0 and it < X.n_routing - 2:
        v.tensor_copy(bb[:], t.bcur[:])


def _stage_B(X, t, it, c):
    """softmax normalize (DVE) + block-diag scatter to cbd (DMA)."""
    if it == X.n_routing - 1:
        return
    nc, mybir = X.nc, X.mybir
    f32, bf16 = X.f32, X.bf16
    v = nc.vector
    dmaqs = [nc.sync, nc.scalar, nc.gpsimd]
    cbd = X.cbds[c]

    c2 = t.c2
    c2n = X.small.tile([128, O, IB], bf16, tag="c2n", bufs=2, name="c2n")
    e5 = (
        c2n[:].bitcast(f32)
        .rearrange("p a b -> p (a b)")
        .rearrange("p (o i) -> p o i", o=5, i=IB)
    )  # [128, 5, 72] carved over c2n's bytes
    v.tensor_add(e5, c2[:, 0:5, :], c2[:, 5:10, :])
    e2t = X.small.tile([128, 2, IB], f32, tag="e2t", bufs=2, name="e2t")
    v.tensor_add(e2t[:], e5[:, 0:2, :], e5[:, 2:4, :])
    rs = X.small.tile([128, IB], f32, tag="rs", bufs=2, name="rs")
    e1 = rs[:]  # carved: e1 is consumed before rs is written
    v.tensor_add(e1, e2t[:, 0, :], e2t[:, 1, :])
    ssum = X.small.tile([128, IB], f32, tag="ssum", bufs=2, name="ssum")
    v.tensor_add(ssum[:], e1, e5[:, 4, :])
    v.reciprocal(rs[:], ssum[:])
    v.tensor_mul(
        c2n[:], c2[:], rs[:].unsqueeze(1).broadcast_to([128, O, IB])
    )
    for b in range(BC):
        dmaqs[b % 3].dma_start(
            out=cbd[b * 16:(b + 1) * 16, b:80:8, :],
            in_=c2n[b * 16:(b + 1) * 16, :, :],
        )


_CACHE = {}


def _get(n_routing: int):
    if n_routing not in _CACHE:
        _CACHE[n_routing] = _build(n_routing)
    return _CACHE[n_routing]


def _bf16(a):
    import ml_dtypes

    return np.asarray(a, dtype=ml_dtypes.bfloat16)


def _prep_host(inputs: np.ndarray, W: np.ndarray):
    x = np.ascontiguousarray(np.asarray(inputs, dtype=np.float32))
    W = np.asarray(W, dtype=np.float32)
    # w2[(il,v), ib, (o,n)] = W[ib*16+il, o, v, n]
    w2 = np.ascontiguousarray(
        W.reshape(IB, 16, O, V, N).transpose(1, 3, 0, 2, 4).reshape(128, IB, ON)
    )
    # e2[(b,il), (o,b')] = 0.1 * (b == b')   (uniform softmax weights)
    e2 = np.zeros((128, 80), dtype=np.float32)
    for b in range(8):
        e2[b * 16:(b + 1) * 16, np.arange(O) * 8 + b] = 0.1
    # ed[b, (b',il)] = (b == b')   (vrep replication stationary)
    ed = np.zeros((8, 128), dtype=np.float32)
    for b in range(8):
        ed[b, b * 16:(b + 1) * 16] = 1.0
    return x, _bf16(w2), _bf16(e2), _bf16(ed)


def _make_in_maps(inputs, W):
    x, w2, e2, ed = _prep_host(inputs, W)
    in_maps = []
    for core in range(NCORES):
        xc = x[core * BLOC:(core + 1) * BLOC]              # [32, 1152, 8]
        # xbdh[c, il*8+v, ib, b*16+il] = xc[c*BC+b, ib*16+il, v]
        xr = xc.reshape(NCHUNK, BC, IB, 16, V)
        xbdh = np.zeros((NCHUNK, 128, IB, 128), dtype=np.float32)
        for il in range(16):
            xbdh[:, il * 8:(il + 1) * 8, :, il::16] = xr[:, :, :, il, :].transpose(
                0, 3, 2, 1
            )
        in_maps.append(
            {"xbdh": _bf16(xbdh), "w2": w2, "e2": e2, "ed": ed}
        )
    return in_maps


def kernel(inputs, W, n_routing):
    from concourse.bass_utils import run_bass_kernel_spmd

    n_routing = int(n_routing)
    nc = _get(n_routing)
    in_maps = _make_in_maps(inputs, W)
    res = run_bass_kernel_spmd(nc, in_maps, core_ids=list(range(NCORES)))
    outs = [res.results[i]["out"] for i in range(NCORES)]
    return np.concatenate(outs, axis=0).astype(np.float32)


# revision 24
# speedup vs baseline: 1.0532x; 1.0532x over previous
"""CapsuleLayer (dynamic routing) Trainium2 Bass kernel.

Math (per example b):
  u_hat[b,i,o,n] = sum_v x[b,i,v] * W[i,o,v,n]        I=1152, O=10, V=8, N=16
  b_logits = 0; repeat n_routing times:
    c = softmax_o(b_logits); s = sum_i c*u_hat; out = squash(s)
    if not last: b_logits += sum_n u_hat*out

Distribution: batch B=256 sharded over 8 cores (32 each). W replicated.

Per-core layout (chunk = 8 examples, 4 chunks), i = ib*16 + il:
  K partitions k = il*8+v   (contraction rows of the u_hat matmul)
  M partitions p = b*16+il  (rows of u_hat / routing state)
  U[c] [128, 72, 160] bf16  u_hat,  U[(b,il), ib, (o,n)]
  xbd  [128, 18, 128] bf16  block-diag x stationary quarters (2 rotating bufs)
  cbd[c] [128, 80, 72] bf16 block-diag c stationary: CBD[(b,il), (o,b'), ib]
  w2   [128, 72, 160] bf16  W2[(il,v), ib, (o,n)] = W[ib*16+il, o, v, n]
  u_hat matmul (per ib): psum[(b,il'),(o,n)] = XBD[:,ib,:].T @ w2[:,ib,:]
  s matmul (per iter): psum[(o,b'),(o',n)] += CBD[:,:,ib].T @ U[:,ib,:]
    -> diagonal o==o' holds s[b', o, n]  (extracted via small DMAs)

Schedule: phase 1 computes u_hat for ALL 4 chunks up front; routing runs as
a SOFTWARE PIPELINE over (iteration, chunk) tasks with three stages
  S(k): s-matmul (PE), PSUM evacuation (ACT), diag extraction (DMA),
        squash front  s^2 + n-tree + (1+nsq)^2*(nsq+eps)  (GPSIMD)
  A(k): rsqrt chain + squash scale + v (DVE, STT-fused Newton), v->bf16,
        vrep replication via a tiny ones-block-diag matmul (PE) + PSUM
        copy (ACT), agreement product + t8/t4 tree (DVE), t2t/final/bsum
        levels (GPSIMD), per-half exp (ACT)
  B(k): softmax o-sum tree + reciprocal + normalize (DVE), block-diag
        scatter to cbd (DMA)
emitted as S(k), A(k-1), B(k-2) so every engine's FIFO only sees work whose
inputs were produced >= 1 task-period earlier - no cross-engine stalls.

SBUF is within ~1KB of full, so scratch is carved aggressively: all squash
scalar temps live in slices of one [8, 248] tile, the softmax o-sum tree is
carved into the c2n/rs tiles via bitcast views, and the agreement t4/t2t
levels are carved back into ph's storage after it is consumed.
"""

import os
import sys

import numpy as np

_TRN_REPO = "/opt/trn_rl_repo"
if _TRN_REPO not in sys.path:
    sys.path.insert(0, _TRN_REPO)

EPS = 1e-10
B, I, V, O, N = 256, 1152, 8, 10, 16
NCORES = 8
BLOC = B // NCORES          # 32 examples per core
BC = 8                      # examples per chunk
NCHUNK = BLOC // BC         # 4
IB = I // 16                # 72 i-blocks
IBH = IB // 2               # 36 (agreement half granularity)
IBQ = IB // 4               # 18 (xbd staging granularity)
ON = O * N                  # 160
RSQRT_MAGIC = 0x5F3759DF


class _Ctx:
    pass


def _build(n_routing: int):
    import concourse.bacc as bacc
    import concourse.tile as tile
    from concourse import mybir

    nc = bacc.Bacc("TRN2", target_bir_lowering=False, debug=False)
    f32 = mybir.dt.float32
    bf16 = mybir.dt.bfloat16

    xbdh = nc.dram_tensor(
        "xbdh", [NCHUNK, 128, IB, 128], bf16, kind="ExternalInput"
    )
    w2 = nc.dram_tensor("w2", [128, IB, ON], bf16, kind="ExternalInput")
    e2 = nc.dram_tensor("e2", [128, 80], bf16, kind="ExternalInput")
    ed = nc.dram_tensor("ed", [8, 128], bf16, kind="ExternalInput")
    out_d = nc.dram_tensor("out", [BLOC, O, N], f32, kind="ExternalOutput")

    X = _Ctx()
    X.nc, X.mybir = nc, mybir
    X.f32, X.bf16 = f32, bf16
    X.i32 = mybir.dt.int32
    X.out_d = out_d
    X.n_routing = n_routing

    with tile.TileContext(nc) as tc:
        with (
            tc.tile_pool(name="state", bufs=1) as state,
            tc.tile_pool(name="small", bufs=1) as small,
            tc.tile_pool(name="tree", bufs=1) as tree,
            tc.tile_pool(name="psA", bufs=3, space="PSUM") as psA,
            tc.tile_pool(name="psS", bufs=4, space="PSUM") as psS,
        ):
            X.small, X.tree, X.psS, X.psA = small, tree, psS, psA
            X.Us = [
                state.tile([128, IB, ON], bf16, tag=f"U{j}", name=f"U{j}")
                for j in range(NCHUNK)
            ]
            X.cbds = [
                state.tile([128, 80, IB], bf16, tag=f"cbd{j}", name=f"cbd{j}")
                for j in range(NCHUNK)
            ] if n_routing > 1 else []
            for j, cb in enumerate(X.cbds):
                if j % 2 == 0:
                    nc.scalar.memzero(cb[:])
                else:
                    nc.gpsimd.memset(cb[:], 0.0)
            X.bbs = [
                state.tile([128, IB, O], bf16, tag=f"bb{j}", name=f"bb{j}")
                for j in range(NCHUNK)
            ] if n_routing > 1 else []
            X.e2s = state.tile([128, 80], bf16, name="e2s")
            nc.sync.dma_start(out=X.e2s[:], in_=e2[:])
            X.eds = state.tile([8, 128], bf16, name="eds")
            nc.sync.dma_start(out=X.eds[:], in_=ed[:])
            w2s = state.tile([128, IB, ON], bf16, name="w2s")
            for q in range(4):
                nc.scalar.dma_start(
                    out=w2s[:, q * IBQ:(q + 1) * IBQ, :],
                    in_=w2[:, q * IBQ:(q + 1) * IBQ, :],
                )
            X.v3b = state.tile([32, ON], bf16, name="v3b")
            if n_routing > 1:
                nc.vector.memset(X.v3b[:], 0.0)
            X.vrep = state.tile([128, ON], bf16, name="vrep")
            # squash-front constants (GPSIMD runs only tensor_tensor ops)
            X.csq = state.tile([BC, 2 * O], f32, name="csq")
            nc.gpsimd.memset(X.csq[:, 0 * O:1 * O], 1.0)
            nc.gpsimd.memset(X.csq[:, 1 * O:2 * O], EPS)

            # ---------------- phase 1: u_hat for all chunks ----------------
            for c in range(NCHUNK):
                for h in range(4):
                    xbd = small.tile(
                        [128, IBQ, 128], bf16, tag="xbd", bufs=2, name="xbd"
                    )
                    nc.sync.dma_start(
                        out=xbd[:], in_=xbdh[c, :, h * IBQ:(h + 1) * IBQ, :]
                    )
                    for g in range(IBQ // 3):
                        ps = psA.tile([128, 3, ON], f32, tag="psA")
                        for j in range(3):
                            ib = h * IBQ + g * 3 + j
                            nc.tensor.matmul(
                                ps[:, j, :],
                                xbd[:, g * 3 + j, :],
                                w2s[:, ib, :],
                                start=True,
                                stop=True,
                            )
                        dst = X.Us[c][
                            :, h * IBQ + g * 3:h * IBQ + (g + 1) * 3, :
                        ]
                        if g % 2 == 0:
                            nc.vector.tensor_copy(dst, ps[:])
                        else:
                            nc.scalar.copy(dst, ps[:])

            # ------- routing: software-pipelined S/A/B over (it, c) -------
            tasks = [(it, c) for it in range(n_routing) for c in range(NCHUNK)]
            ctxs = [None] * len(tasks)
            for k in range(len(tasks) + 2):
                if k < len(tasks):
                    ctxs[k] = _stage_S(X, *tasks[k])
                if 1 <= k <= len(tasks):
                    _stage_A(X, ctxs[k - 1], *tasks[k - 1])
                if k >= 2:
                    _stage_B(X, ctxs[k - 2], *tasks[k - 2])

    nc.compile()
    return nc


def _stage_S(X, it, c):
    """s-matmul (PE) -> sY (ACT) -> diag extract (DMA) -> squash front
    (GPSIMD: s^2, n-tree, (1+nsq)^2*(nsq+eps))."""
    nc, mybir = X.nc, X.mybir
    f32, N_ = X.f32, N
    g = nc.gpsimd
    OP = mybir.AluOpType
    dmaqs = [nc.sync, nc.scalar, nc.gpsimd]
    U, cbd = X.Us[c], (X.cbds[c] if X.cbds else None)

    t = _Ctx()
    pss = X.psS.tile([80, ON], f32, tag="psS")
    for ib in range(IB):
        lhsT = X.e2s[:] if it == 0 else cbd[:, :, ib]
        nc.tensor.matmul(
            pss[:], lhsT, U[:, ib, :], start=(ib == 0), stop=(ib == IB - 1)
        )
    sY = X.small.tile([80, ON], f32, tag="sY", bufs=2)
    nc.scalar.copy(sY[:], pss[:])
    t.s3 = X.small.tile([BC, ON], f32, tag="s3", bufs=2)
    for o in range(O):
        dmaqs[o % 3].dma_start(
            out=t.s3[:, o * N_:(o + 1) * N_],
            in_=sY[o * 8:(o + 1) * 8, o * N_:(o + 1) * N_],
        )
    # squash front: GPSIMD while DVE is busy with agreement; in the last
    # iteration DVE is idle, and 4 serialized GPSIMD fronts would gate the
    # tail, so it runs on DVE there (nsq via one 1x tensor_reduce).
    sw = X.small.tile([BC, 248], f32, tag="sw", bufs=2, name="sw")
    t.sw = sw
    sq = sw[:, 0:160].rearrange("b (o n) -> b o n", n=N_)
    q8 = sw[:, 160:240].rearrange("b (o n) -> b o n", n=8)
    q4 = sw[:, 0:40].rearrange("b (o n) -> b o n", n=4)
    q2 = sw[:, 40:60].rearrange("b (o n) -> b o n", n=2)
    nsq = sw[:, 60:70]
    np1 = sw[:, 70:80]
    d1 = sw[:, 80:90]
    dd = sw[:, 90:100]
    nse = sw[:, 110:120]
    s34 = t.s3[:].rearrange("b (o n) -> b o n", n=N_)
    if it == X.n_routing - 1:
        v = nc.vector
        AX = mybir.AxisListType
        v.tensor_mul(sq, s34, s34)
        v.tensor_reduce(nsq.rearrange("b (o u) -> b o u", u=1), sq,
                        axis=AX.X, op=OP.add)
        v.tensor_scalar_add(np1, nsq, 1.0)
        v.tensor_mul(d1, np1, np1)
        v.scalar_tensor_tensor(dd, nsq, EPS, d1, op0=OP.add, op1=OP.mult)
    else:
        g.tensor_mul(sq, s34, s34)
        g.tensor_add(q8, sq[:, :, 0:8], sq[:, :, 8:16])
        g.tensor_add(q4, q8[:, :, 0:4], q8[:, :, 4:8])
        g.tensor_add(q2, q4[:, :, 0:2], q4[:, :, 2:4])
        g.tensor_add(nsq, q2[:, :, 0], q2[:, :, 1])
        g.tensor_add(np1, nsq, X.csq[:, 0:O])
        g.tensor_mul(d1, np1, np1)
        g.tensor_add(nse, nsq, X.csq[:, O:2 * O])
        g.tensor_mul(dd, d1, nse)
    return t


def _stage_A(X, t, it, c):
    """DVE rsqrt chain + v3; vrep via ones-matmul (PE) + copy (ACT);
    agreement products + t8/t4 (DVE); t2t/final/bsum (GPSIMD); exp (ACT)."""
    nc, mybir = X.nc, X.mybir
    f32, bf16, i32, N_ = X.f32, X.bf16, X.i32, N
    OP = mybir.AluOpType
    AF = mybir.ActivationFunctionType
    v = nc.vector
    g = nc.gpsimd
    U, bb = X.Us[c], (X.bbs[c] if X.bbs else None)
    last = it == X.n_routing - 1

    sw = t.sw
    nsq = sw[:, 60:70]
    dd = sw[:, 90:100]
    ya = sw[:, 100:110]
    y2 = sw[:, 110:120]     # nse slot, consumed
    w1 = sw[:, 120:130]
    yb = sw[:, 140:150]
    v.tensor_scalar(
        ya.bitcast(i32), dd.bitcast(i32), 1, None,
        op0=OP.logical_shift_right,
    )
    v.tensor_scalar(
        ya.bitcast(i32), ya.bitcast(i32), -1, RSQRT_MAGIC,
        op0=OP.mult, op1=OP.add,
    )
    yy, yn = ya, yb
    for _ in range(2):
        v.tensor_mul(y2, yy, yy)
        v.scalar_tensor_tensor(w1, y2, -0.5, dd, op0=OP.mult, op1=OP.mult)
        v.scalar_tensor_tensor(yn, w1, 1.5, yy, op0=OP.add, op1=OP.mult)
        yy, yn = yn, yy
    sc = y2
    v.tensor_mul(sc, nsq, yy)
    v3 = X.small.tile([BC, ON], f32, tag="v3", bufs=2)
    v.tensor_mul(
        v3[:].rearrange("b (o n) -> b o n", n=N_),
        t.s3[:].rearrange("b (o n) -> b o n", n=N_),
        sc.unsqueeze(2).broadcast_to([BC, O, N_]),
    )

    if last:
        nc.scalar.dma_start(
            out=X.out_d[c * BC:(c + 1) * BC, :, :],
            in_=v3[:].rearrange("b (o n) -> b o n", n=N_),
        )
        return

    # vrep[(b,il), (o,n)] = v3[b, (o,n)] via ones-block-diag matmul
    v.tensor_copy(X.v3b[0:BC, :], v3[:])
    psv = X.psA.tile([128, ON], f32, tag="psv", bufs=1, name="psv")
    nc.tensor.matmul(psv[:], X.eds[:], X.v3b[0:BC, :], start=True, stop=True)
    nc.scalar.copy(X.vrep[:], psv[:])

    # agreement product + tree; t2t/final/bsum levels go to GPSIMD
    t.bcur = bb if it == 0 else X.small.tile(
        [128, IB, O], bf16, tag="bsum", bufs=2, name="bsum"
    )
    t.c2 = X.small.tile([128, O, IB], bf16, tag="c2", bufs=2, name="c2")
    for h in range(2):
        ph = X.tree.tile([128, IBH * ON], bf16, tag="ph", name="ph")
        phv = ph[:].rearrange("p (i o n) -> p i o n", o=O, n=N_)
        t4v = ph[:, 0:IBH * O * 4].rearrange(
            "p (i o n) -> p i o n", o=O, n=4
        )
        t2v = ph[:, IBH * O * 4:IBH * O * 6].rearrange(
            "p (i o n) -> p i o n", o=O, n=2
        )
        afv = ph[:, IBH * O * 6:IBH * O * 7].rearrange(
            "p (i o) -> p i o", o=O
        )
        v.tensor_mul(
            phv,
            U[:, h * IBH:(h + 1) * IBH, :].rearrange(
                "p i (o n) -> p i o n", n=N_
            ),
            X.vrep[:]
            .rearrange("p (o n) -> p o n", n=N_)
            .unsqueeze(1)
            .broadcast_to([128, IBH, O, N_]),
        )
        t8 = X.tree.tile([128, IBH, O, 8], bf16, tag="t8", name="t8")
        v.tensor_add(t8[:], phv[:, :, :, 0:8], phv[:, :, :, 8:16])
        v.tensor_add(t4v, t8[:, :, :, 0:4], t8[:, :, :, 4:8])
        v.tensor_add(t2v, t4v[:, :, :, 0:2], t4v[:, :, :, 2:4])
        bslice = t.bcur[:, h * IBH:(h + 1) * IBH, :]
        if it == 0:
            v.tensor_add(bslice, t2v[:, :, :, 0], t2v[:, :, :, 1])
        else:
            v.tensor_add(afv, t2v[:, :, :, 0], t2v[:, :, :, 1])
            v.tensor_add(bslice, afv, bb[:, h * IBH:(h + 1) * IBH, :])
        # exp of this half on ACT as soon as its logits land
        nc.scalar.activation(
            t.c2[:, :, h * IBH:(h + 1) * IBH].transpose([0, 2, 1]),
            bslice, AF.Exp,
        )
    if it != 0 and it < X.n_routing - 2:
        v.tensor_copy(bb[:], t.bcur[:])


def _stage_B(X, t, it, c):
    """softmax normalize (DVE) + block-diag scatter to cbd (DMA)."""
    if it == X.n_routing - 1:
        return
    nc, mybir = X.nc, X.mybir
    f32, bf16 = X.f32, X.bf16
    v = nc.vector
    dmaqs = [nc.sync, nc.scalar, nc.gpsimd]
    cbd = X.cbds[c]

    c2 = t.c2
    c2n = X.small.tile([128, O, IB], bf16, tag="c2n", bufs=2, name="c2n")
    e5 = (
        c2n[:].bitcast(f32)
        .rearrange("p a b -> p (a b)")
        .rearrange("p (o i) -> p o i", o=5, i=IB)
    )  # [128, 5, 72] carved over c2n's bytes
    v.tensor_add(e5, c2[:, 0:5, :], c2[:, 5:10, :])
    e2t = X.small.tile([128, 2, IB], f32, tag="e2t", bufs=2, name="e2t")
    v.tensor_add(e2t[:], e5[:, 0:2, :], e5[:, 2:4, :])
    rs = X.small.tile([128, IB], f32, tag="rs", bufs=2, name="rs")
    e1 = rs[:]  # carved: e1 is consumed before rs is written
    v.tensor_add(e1, e2t[:, 0, :], e2t[:, 1, :])
    ssum = X.small.tile([128, IB], f32, tag="ssum", bufs=2, name="ssum")
    v.tensor_add(ssum[:], e1, e5[:, 4, :])
    v.reciprocal_approx_accurate(rs[:], ssum[:], scratch=e2t[:, 1, :])
    v.tensor_mul(
        c2n[:], c2[:], rs[:].unsqueeze(1).broadcast_to([128, O, IB])
    )
    for b in range(BC):
        dmaqs[b % 3].dma_start(
            out=cbd[b * 16:(b + 1) * 16, b:80:8, :],
            in_=c2n[b * 16:(b + 1) * 16, :, :],
        )


_CACHE = {}


def _get(n_routing: int):
    if n_routing not in _CACHE:
        _CACHE[n_routing] = _build(n_routing)
    return _CACHE[n_routing]


def _bf16(a):
    import ml_dtypes

    return np.asarray(a, dtype=ml_dtypes.bfloat16)


def _prep_host(inputs: np.ndarray, W: np.ndarray):
    x = np.ascontiguousarray(np.asarray(inputs, dtype=np.float32))
    W = np.asarray(W, dtype=np.float32)
    # w2[(il,v), ib, (o,n)] = W[ib*16+il, o, v, n]
    w2 = np.ascontiguousarray(
        W.reshape(IB, 16, O, V, N).transpose(1, 3, 0, 2, 4).reshape(128, IB, ON)
    )
    # e2[(b,il), (o,b')] = 0.1 * (b == b')   (uniform softmax weights)
    e2 = np.zeros((128, 80), dtype=np.float32)
    for b in range(8):
        e2[b * 16:(b + 1) * 16, np.arange(O) * 8 + b] = 0.1
    # ed[b, (b',il)] = (b == b')   (vrep replication stationary)
    ed = np.zeros((8, 128), dtype=np.float32)
    for b in range(8):
        ed[b, b * 16:(b + 1) * 16] = 1.0
    return x, _bf16(w2), _bf16(e2), _bf16(ed)


def _make_in_maps(inputs, W):
    x, w2, e2, ed = _prep_host(inputs, W)
    in_maps = []
    for core in range(NCORES):
        xc = x[core * BLOC:(core + 1) * BLOC]              # [32, 1152, 8]
        # xbdh[c, il*8+v, ib, b*16+il] = xc[c*BC+b, ib*16+il, v]
        xr = xc.reshape(NCHUNK, BC, IB, 16, V)
        xbdh = np.zeros((NCHUNK, 128, IB, 128), dtype=np.float32)
        for il in range(16):
            xbdh[:, il * 8:(il + 1) * 8, :, il::16] = xr[:, :, :, il, :].transpose(
                0, 3, 2, 1
            )
        in_maps.append(
            {"xbdh": _bf16(xbdh), "w2": w2, "e2": e2, "ed": ed}
        )
    return in_maps


def kernel(inputs, W, n_routing):
    from concourse.bass_utils import run_bass_kernel_spmd

    n_routing = int(n_routing)
    nc = _get(n_routing)
    in_maps = _make_in_maps(inputs, W)
    res = run_bass_kernel_spmd(nc, in_maps, core_ids=list(range(NCORES)))
    outs = [res.results[i]["out"] for i in range(NCORES)]
    return np.concatenate(outs, axis=0).astype(np.float32)


# revision 25
# speedup vs baseline: 1.0896x; 1.0346x over previous
"""CapsuleLayer (dynamic routing) Trainium2 Bass kernel.

Math (per example b):
  u_hat[b,i,o,n] = sum_v x[b,i,v] * W[i,o,v,n]        I=1152, O=10, V=8, N=16
  b_logits = 0; repeat n_routing times:
    c = softmax_o(b_logits); s = sum_i c*u_hat; out = squash(s)
    if not last: b_logits += sum_n u_hat*out

Distribution: batch B=256 sharded over 8 cores (32 each). W replicated.

Per-core layout (chunk = 8 examples, 4 chunks), i = ib*16 + il:
  K partitions k = il*8+v   (contraction rows of the u_hat matmul)
  M partitions p = b*16+il  (rows of u_hat / routing state)
  U[c] [128, 72, 160] bf16  u_hat,  U[(b,il), ib, (o,n)]
  xbd  [128, 18, 128] bf16  block-diag x stationary quarters (2 rotating bufs)
  cbd[c] [128, 80, 72] bf16 block-diag c stationary: CBD[(b,il), (o,b'), ib]
  w2   [128, 72, 160] bf16  W2[(il,v), ib, (o,n)] = W[ib*16+il, o, v, n]
  u_hat matmul (per ib): psum[(b,il'),(o,n)] = XBD[:,ib,:].T @ w2[:,ib,:]
  s matmul (per iter): psum[(o,b'),(o',n)] += CBD[:,:,ib].T @ U[:,ib,:]
    -> diagonal o==o' holds s[b', o, n]  (extracted via small DMAs)

Schedule: phase 1 computes u_hat for ALL 4 chunks up front; routing runs as
a SOFTWARE PIPELINE over (iteration, chunk) tasks with three stages
  S(k): s-matmul (PE), PSUM evacuation (ACT), diag extraction (DMA),
        squash front  s^2 + n-tree + (1+nsq)^2*(nsq+eps)  (GPSIMD)
  A(k): rsqrt chain + squash scale + v (DVE, STT-fused Newton), v->bf16,
        vrep replication via a tiny ones-block-diag matmul (PE) + PSUM
        copy (ACT), agreement product + t8/t4 tree (DVE), t2t/final/bsum
        levels (GPSIMD), per-half exp (ACT)
  B(k): softmax o-sum tree + reciprocal + normalize (DVE), block-diag
        scatter to cbd (DMA)
emitted as S(k), A(k-1), B(k-2) so every engine's FIFO only sees work whose
inputs were produced >= 1 task-period earlier - no cross-engine stalls.

SBUF is within ~1KB of full, so scratch is carved aggressively: all squash
scalar temps live in slices of one [8, 248] tile, the softmax o-sum tree is
carved into the c2n/rs tiles via bitcast views, and the agreement t4/t2t
levels are carved back into ph's storage after it is consumed.
"""

import os
import sys

import numpy as np

_TRN_REPO = "/opt/trn_rl_repo"
if _TRN_REPO not in sys.path:
    sys.path.insert(0, _TRN_REPO)

EPS = 1e-10
B, I, V, O, N = 256, 1152, 8, 10, 16
NCORES = 8
BLOC = B // NCORES          # 32 examples per core
BC = 8                      # examples per chunk
NCHUNK = BLOC // BC         # 4
IB = I // 16                # 72 i-blocks
IBH = IB // 2               # 36 (agreement half granularity)
IBQ = IB // 4               # 18 (xbd staging granularity)
ON = O * N                  # 160
RSQRT_MAGIC = 0x5F3759DF


class _Ctx:
    pass


def _build(n_routing: int):
    import concourse.bacc as bacc
    import concourse.tile as tile
    from concourse import mybir

    nc = bacc.Bacc("TRN2", target_bir_lowering=False, debug=False)
    f32 = mybir.dt.float32
    bf16 = mybir.dt.bfloat16

    xbdh = nc.dram_tensor(
        "xbdh", [NCHUNK, 128, IB, 128], bf16, kind="ExternalInput"
    )
    w2 = nc.dram_tensor("w2", [128, IB, ON], bf16, kind="ExternalInput")
    e2 = nc.dram_tensor("e2", [128, 80], bf16, kind="ExternalInput")
    ed = nc.dram_tensor("ed", [8, 128], bf16, kind="ExternalInput")
    out_d = nc.dram_tensor("out", [BLOC, O, N], f32, kind="ExternalOutput")

    X = _Ctx()
    X.nc, X.mybir = nc, mybir
    X.f32, X.bf16 = f32, bf16
    X.i32 = mybir.dt.int32
    X.out_d = out_d
    X.n_routing = n_routing

    with tile.TileContext(nc) as tc:
        with (
            tc.tile_pool(name="state", bufs=1) as state,
            tc.tile_pool(name="small", bufs=1) as small,
            tc.tile_pool(name="tree", bufs=1) as tree,
            tc.tile_pool(name="psA", bufs=3, space="PSUM") as psA,
            tc.tile_pool(name="psS", bufs=4, space="PSUM") as psS,
        ):
            X.small, X.tree, X.psS, X.psA = small, tree, psS, psA
            X.Us = [
                state.tile([128, IB, ON], bf16, tag=f"U{j}", name=f"U{j}")
                for j in range(NCHUNK)
            ]
            X.cbds = [
                state.tile([128, 80, IB], bf16, tag=f"cbd{j}", name=f"cbd{j}")
                for j in range(NCHUNK)
            ] if n_routing > 1 else []
            for j, cb in enumerate(X.cbds):
                if j % 2 == 0:
                    nc.scalar.memzero(cb[:])
                else:
                    nc.gpsimd.memset(cb[:], 0.0)
            X.bbs = [
                state.tile([128, IB, O], bf16, tag=f"bb{j}", name=f"bb{j}")
                for j in range(NCHUNK)
            ] if n_routing > 1 else []
            X.e2s = state.tile([128, 80], bf16, name="e2s")
            nc.sync.dma_start(out=X.e2s[:], in_=e2[:])
            X.eds = state.tile([8, 128], bf16, name="eds")
            nc.sync.dma_start(out=X.eds[:], in_=ed[:])
            w2s = state.tile([128, IB, ON], bf16, name="w2s")
            for q in range(4):
                nc.scalar.dma_start(
                    out=w2s[:, q * IBQ:(q + 1) * IBQ, :],
                    in_=w2[:, q * IBQ:(q + 1) * IBQ, :],
                )
            X.v3b = state.tile([32, ON], bf16, name="v3b")
            if n_routing > 1:
                nc.vector.memset(X.v3b[:], 0.0)
            X.vrep = state.tile([128, ON], bf16, name="vrep")
            # squash-front constants (GPSIMD runs only tensor_tensor ops)
            X.csq = state.tile([BC, 2 * O], f32, name="csq")
            nc.gpsimd.memset(X.csq[:, 0 * O:1 * O], 1.0)
            nc.gpsimd.memset(X.csq[:, 1 * O:2 * O], EPS)

            # ------- phase 1 (u_hat), with it0's S-stage interleaved -------
            tasks = [(it, c) for it in range(n_routing) for c in range(NCHUNK)]
            ctxs = [None] * len(tasks)
            for c in range(NCHUNK):
                for h in range(4):
                    xbd = small.tile(
                        [128, IBQ, 128], bf16, tag="xbd", bufs=2, name="xbd"
                    )
                    nc.sync.dma_start(
                        out=xbd[:], in_=xbdh[c, :, h * IBQ:(h + 1) * IBQ, :]
                    )
                    for g in range(IBQ // 3):
                        ps = psA.tile([128, 3, ON], f32, tag="psA")
                        for j in range(3):
                            ib = h * IBQ + g * 3 + j
                            nc.tensor.matmul(
                                ps[:, j, :],
                                xbd[:, g * 3 + j, :],
                                w2s[:, ib, :],
                                start=True,
                                stop=True,
                            )
                        dst = X.Us[c][
                            :, h * IBQ + g * 3:h * IBQ + (g + 1) * 3, :
                        ]
                        if g % 2 == 0:
                            nc.vector.tensor_copy(dst, ps[:])
                        else:
                            nc.scalar.copy(dst, ps[:])
                # chunk c's u_hat is complete: issue its it0 s-matmul now
                ctxs[c] = _stage_S(X, 0, c)

            # ------- routing: software-pipelined S/A/B over (it, c) -------
            for k in range(len(tasks) + 2):
                if k < len(tasks) and k >= NCHUNK:
                    ctxs[k] = _stage_S(X, *tasks[k])
                if 1 <= k <= len(tasks):
                    _stage_A(X, ctxs[k - 1], *tasks[k - 1])
                if k >= 2:
                    _stage_B(X, ctxs[k - 2], *tasks[k - 2])

    nc.compile()
    return nc


def _stage_S(X, it, c):
    """s-matmul (PE) -> sY (ACT) -> diag extract (DMA) -> squash front
    (GPSIMD: s^2, n-tree, (1+nsq)^2*(nsq+eps))."""
    nc, mybir = X.nc, X.mybir
    f32, N_ = X.f32, N
    g = nc.gpsimd
    OP = mybir.AluOpType
    dmaqs = [nc.sync, nc.scalar, nc.gpsimd]
    U, cbd = X.Us[c], (X.cbds[c] if X.cbds else None)

    t = _Ctx()
    pss = X.psS.tile([80, ON], f32, tag="psS")
    for ib in range(IB):
        lhsT = X.e2s[:] if it == 0 else cbd[:, :, ib]
        nc.tensor.matmul(
            pss[:], lhsT, U[:, ib, :], start=(ib == 0), stop=(ib == IB - 1)
        )
    sY = X.small.tile([80, ON], f32, tag="sY", bufs=2)
    nc.scalar.copy(sY[:], pss[:])
    t.s3 = X.small.tile([BC, ON], f32, tag="s3", bufs=2)
    for o in range(O):
        dmaqs[o % 3].dma_start(
            out=t.s3[:, o * N_:(o + 1) * N_],
            in_=sY[o * 8:(o + 1) * 8, o * N_:(o + 1) * N_],
        )
    # squash front: GPSIMD while DVE is busy with agreement; in the last
    # iteration DVE is idle, and 4 serialized GPSIMD fronts would gate the
    # tail, so it runs on DVE there (nsq via one 1x tensor_reduce).
    sw = X.small.tile([BC, 248], f32, tag="sw", bufs=2, name="sw")
    t.sw = sw
    sq = sw[:, 0:160].rearrange("b (o n) -> b o n", n=N_)
    q8 = sw[:, 160:240].rearrange("b (o n) -> b o n", n=8)
    q4 = sw[:, 0:40].rearrange("b (o n) -> b o n", n=4)
    q2 = sw[:, 40:60].rearrange("b (o n) -> b o n", n=2)
    nsq = sw[:, 60:70]
    np1 = sw[:, 70:80]
    d1 = sw[:, 80:90]
    dd = sw[:, 90:100]
    nse = sw[:, 110:120]
    s34 = t.s3[:].rearrange("b (o n) -> b o n", n=N_)
    if it == X.n_routing - 1:
        v = nc.vector
        AX = mybir.AxisListType
        v.tensor_mul(sq, s34, s34)
        v.tensor_reduce(nsq.rearrange("b (o u) -> b o u", u=1), sq,
                        axis=AX.X, op=OP.add)
        v.tensor_scalar_add(np1, nsq, 1.0)
        v.tensor_mul(d1, np1, np1)
        v.scalar_tensor_tensor(dd, nsq, EPS, d1, op0=OP.add, op1=OP.mult)
    else:
        g.tensor_mul(sq, s34, s34)
        g.tensor_add(q8, sq[:, :, 0:8], sq[:, :, 8:16])
        g.tensor_add(q4, q8[:, :, 0:4], q8[:, :, 4:8])
        g.tensor_add(q2, q4[:, :, 0:2], q4[:, :, 2:4])
        g.tensor_add(nsq, q2[:, :, 0], q2[:, :, 1])
        g.tensor_add(np1, nsq, X.csq[:, 0:O])
        g.tensor_mul(d1, np1, np1)
        g.tensor_add(nse, nsq, X.csq[:, O:2 * O])
        g.tensor_mul(dd, d1, nse)
    return t


def _stage_A(X, t, it, c):
    """DVE rsqrt chain + v3; vrep via ones-matmul (PE) + copy (ACT);
    agreement products + t8/t4 (DVE); t2t/final/bsum (GPSIMD); exp (ACT)."""
    nc, mybir = X.nc, X.mybir
    f32, bf16, i32, N_ = X.f32, X.bf16, X.i32, N
    OP = mybir.AluOpType
    AF = mybir.ActivationFunctionType
    v = nc.vector
    g = nc.gpsimd
    U, bb = X.Us[c], (X.bbs[c] if X.bbs else None)
    last = it == X.n_routing - 1

    sw = t.sw
    nsq = sw[:, 60:70]
    dd = sw[:, 90:100]
    ya = sw[:, 100:110]
    y2 = sw[:, 110:120]     # nse slot, consumed
    w1 = sw[:, 120:130]
    yb = sw[:, 140:150]
    v.tensor_scalar(
        ya.bitcast(i32), dd.bitcast(i32), 1, None,
        op0=OP.logical_shift_right,
    )
    v.tensor_scalar(
        ya.bitcast(i32), ya.bitcast(i32), -1, RSQRT_MAGIC,
        op0=OP.mult, op1=OP.add,
    )
    yy, yn = ya, yb
    for _ in range(2):
        v.tensor_mul(y2, yy, yy)
        v.scalar_tensor_tensor(w1, y2, -0.5, dd, op0=OP.mult, op1=OP.mult)
        v.scalar_tensor_tensor(yn, w1, 1.5, yy, op0=OP.add, op1=OP.mult)
        yy, yn = yn, yy
    sc = y2
    v.tensor_mul(sc, nsq, yy)
    v3 = X.small.tile([BC, ON], f32, tag="v3", bufs=2)
    v.tensor_mul(
        v3[:].rearrange("b (o n) -> b o n", n=N_),
        t.s3[:].rearrange("b (o n) -> b o n", n=N_),
        sc.unsqueeze(2).broadcast_to([BC, O, N_]),
    )

    if last:
        nc.scalar.dma_start(
            out=X.out_d[c * BC:(c + 1) * BC, :, :],
            in_=v3[:].rearrange("b (o n) -> b o n", n=N_),
        )
        return

    # vrep[(b,il), (o,n)] = v3[b, (o,n)] via ones-block-diag matmul
    v.tensor_copy(X.v3b[0:BC, :], v3[:])
    psv = X.psA.tile([128, ON], f32, tag="psv", bufs=1, name="psv")
    nc.tensor.matmul(psv[:], X.eds[:], X.v3b[0:BC, :], start=True, stop=True)
    nc.scalar.copy(X.vrep[:], psv[:])

    # agreement product + tree; t2t/final/bsum levels go to GPSIMD
    t.bcur = bb if it == 0 else X.small.tile(
        [128, IB, O], bf16, tag="bsum", bufs=2, name="bsum"
    )
    t.c2 = X.small.tile([128, O, IB], bf16, tag="c2", bufs=2, name="c2")
    for h in range(2):
        ph = X.tree.tile([128, IBH * ON], bf16, tag="ph", name="ph")
        phv = ph[:].rearrange("p (i o n) -> p i o n", o=O, n=N_)
        t4v = ph[:, 0:IBH * O * 4].rearrange(
            "p (i o n) -> p i o n", o=O, n=4
        )
        t2v = ph[:, IBH * O * 4:IBH * O * 6].rearrange(
            "p (i o n) -> p i o n", o=O, n=2
        )
        afv = ph[:, IBH * O * 6:IBH * O * 7].rearrange(
            "p (i o) -> p i o", o=O
        )
        v.tensor_mul(
            phv,
            U[:, h * IBH:(h + 1) * IBH, :].rearrange(
                "p i (o n) -> p i o n", n=N_
            ),
            X.vrep[:]
            .rearrange("p (o n) -> p o n", n=N_)
            .unsqueeze(1)
            .broadcast_to([128, IBH, O, N_]),
        )
        t8 = X.tree.tile([128, IBH, O, 8], bf16, tag="t8", name="t8")
        v.tensor_add(t8[:], phv[:, :, :, 0:8], phv[:, :, :, 8:16])
        v.tensor_add(t4v, t8[:, :, :, 0:4], t8[:, :, :, 4:8])
        v.tensor_add(t2v, t4v[:, :, :, 0:2], t4v[:, :, :, 2:4])
        bslice = t.bcur[:, h * IBH:(h + 1) * IBH, :]
        if it == 0:
            v.tensor_add(bslice, t2v[:, :, :, 0], t2v[:, :, :, 1])
        else:
            v.tensor_add(afv, t2v[:, :, :, 0], t2v[:, :, :, 1])
            v.tensor_add(bslice, afv, bb[:, h * IBH:(h + 1) * IBH, :])
        # exp of this half on ACT as soon as its logits land
        nc.scalar.activation(
            t.c2[:, :, h * IBH:(h + 1) * IBH].transpose([0, 2, 1]),
            bslice, AF.Exp,
        )
    if it != 0 and it < X.n_routing - 2:
        v.tensor_copy(bb[:], t.bcur[:])


def _stage_B(X, t, it, c):
    """softmax normalize (DVE) + block-diag scatter to cbd (DMA)."""
    if it == X.n_routing - 1:
        return
    nc, mybir = X.nc, X.mybir
    f32, bf16 = X.f32, X.bf16
    v = nc.vector
    dmaqs = [nc.sync, nc.scalar, nc.gpsimd]
    cbd = X.cbds[c]

    c2 = t.c2
    c2n = X.small.tile([128, O, IB], bf16, tag="c2n", bufs=2, name="c2n")
    e5 = (
        c2n[:].bitcast(f32)
        .rearrange("p a b -> p (a b)")
        .rearrange("p (o i) -> p o i", o=5, i=IB)
    )  # [128, 5, 72] carved over c2n's bytes
    v.tensor_add(e5, c2[:, 0:5, :], c2[:, 5:10, :])
    e2t = X.small.tile([128, 2, IB], f32, tag="e2t", bufs=2, name="e2t")
    v.tensor_add(e2t[:], e5[:, 0:2, :], e5[:, 2:4, :])
    rs = X.small.tile([128, IB], f32, tag="rs", bufs=2, name="rs")
    e1 = rs[:]  # carved: e1 is consumed before rs is written
    v.tensor_add(e1, e2t[:, 0, :], e2t[:, 1, :])
    ssum = X.small.tile([128, IB], f32, tag="ssum", bufs=2, name="ssum")
    v.tensor_add(ssum[:], e1, e5[:, 4, :])
    v.reciprocal_approx_accurate(rs[:], ssum[:], scratch=e2t[:, 1, :])
    v.tensor_mul(
        c2n[:], c2[:], rs[:].unsqueeze(1).broadcast_to([128, O, IB])
    )
    for b in range(BC):
        dmaqs[b % 3].dma_start(
            out=cbd[b * 16:(b + 1) * 16, b:80:8, :],
            in_=c2n[b * 16:(b + 1) * 16, :, :],
        )


_CACHE = {}


def _get(n_routing: int):
    if n_routing not in _CACHE:
        _CACHE[n_routing] = _build(n_routing)
    return _CACHE[n_routing]


def _bf16(a):
    import ml_dtypes

    return np.asarray(a, dtype=ml_dtypes.bfloat16)


def _prep_host(inputs: np.ndarray, W: np.ndarray):
    x = np.ascontiguousarray(np.asarray(inputs, dtype=np.float32))
    W = np.asarray(W, dtype=np.float32)
    # w2[(il,v), ib, (o,n)] = W[ib*16+il, o, v, n]
    w2 = np.ascontiguousarray(
        W.reshape(IB, 16, O, V, N).transpose(1, 3, 0, 2, 4).reshape(128, IB, ON)
    )
    # e2[(b,il), (o,b')] = 0.1 * (b == b')   (uniform softmax weights)
    e2 = np.zeros((128, 80), dtype=np.float32)
    for b in range(8):
        e2[b * 16:(b + 1) * 16, np.arange(O) * 8 + b] = 0.1
    # ed[b, (b',il)] = (b == b')   (vrep replication stationary)
    ed = np.zeros((8, 128), dtype=np.float32)
    for b in range(8):
        ed[b, b * 16:(b + 1) * 16] = 1.0
    return x, _bf16(w2), _bf16(e2), _bf16(ed)


def _make_in_maps(inputs, W):
    x, w2, e2, ed = _prep_host(inputs, W)
    in_maps = []
    for core in range(NCORES):
        xc = x[core * BLOC:(core + 1) * BLOC]              # [32, 1152, 8]
        # xbdh[c, il*8+v, ib, b*16+il] = xc[c*BC+b, ib*16+il, v]
        xr = xc.reshape(NCHUNK, BC, IB, 16, V)
        xbdh = np.zeros((NCHUNK, 128, IB, 128), dtype=np.float32)
        for il in range(16):
            xbdh[:, il * 8:(il + 1) * 8, :, il::16] = xr[:, :, :, il, :].transpose(
                0, 3, 2, 1
            )
        in_maps.append(
            {"xbdh": _bf16(xbdh), "w2": w2, "e2": e2, "ed": ed}
        )
    return in_maps


def kernel(inputs, W, n_routing):
    from concourse.bass_utils import run_bass_kernel_spmd

    n_routing = int(n_routing)
    nc = _get(n_routing)
    in_maps = _make_in_maps(inputs, W)
    res = run_bass_kernel_spmd(nc, in_maps, core_ids=list(range(NCORES)))
    outs = [res.results[i]["out"] for i in range(NCORES)]
    return np.concatenate(outs, axis=0).astype(np.float32)
